# revision 1
# baseline (speedup 1.0000x reference)
"""Trainium2 Bass kernel for LinearAttention4 (self-contained).

Problem (per sample): x [256, 56, 56] fp32
  qk = elu(conv1x1(x; qk_w, qk_b)) + 1 ; q, k = split(qk)
  kv = k @ v.T / n ; num = q.T @ kv ; den = q.T @ mean(k) + 1e-6
  attn = (num / den).T ; out = attn + depthwise3x3(x; pe_w) + pe_b

Sharding: data-parallel over batch, 4 samples per core on 8 NeuronCores.
All matmuls run as float32r (TF32-like, 1 cyc/row at N>=256).

Everything on-chip lives in PADDED spatial coordinates (58x58 zero-padded
grid, flattened to 3364 per channel block) so that every matmul operand is
a contiguous 1-free-dim AP (HW requirement). The 3x3 depthwise conv taps
are then pure offsets +-{58,1} into the padded buffer; pad columns yield
garbage outputs which the PSUM->SBUF evacuation AP skips. The kv
contraction runs over all 3364 padded positions: x's pad positions are
zero so they contribute nothing (k's pad columns are explicitly zeroed).

Per-core pipeline (per sample):
  A) DMA host-padded x [2, 128, 3364] -> SBUF
  B) qk matmul (lhsT = qk_w.T chunks, rhs = x spans of 464) + elu+1 via
     min(exp(z+b),1) + relu(z+b)  [exact identity for elu(z+b)+1]
  C) PE-transpose x and k in 116-wide chunks; kv = kT.T @ xT (+ k_sum via
     an N=1 matmul against ones into col 256 of the same psum tile)
  D) den = k_sum.T @ q per span; fold [1,3364]->[116,29] via DMA, +eps,
     reciprocal, unfold, gpsimd partition_broadcast; q *= recip
     (exact: the den scale commutes past the kv contraction)
  E) num matmul + 9 diagonal conv-tap matmuls accumulate into ONE psum
     tile per (c-block, span); ACT evacuates psum + pe_b -> out,
     compacting padded coords back to dense 56x56
"""

import numpy as np

import concourse.bass as bass
import concourse.mybir as mybir
from concourse.tile import TileContext
from concourse.bass_utils import run_bass_kernel_spmd

F32 = mybir.dt.float32
F32R = mybir.dt.float32r

B, C, H, W = 32, 256, 56, 56
N = H * W  # 3136
NCORES = 8
SPC = B // NCORES  # 4
HP = H + 2  # 58
NP = HP * HP  # 3364
SPAN = 8 * HP  # 464 cols per qk/num/conv chunk (8 padded rows)
NCH = 7  # chunks of 8 interior rows
TCH = 116  # transpose chunk width (3364 = 29 * 116)
NTC = NP // TCH  # 29
EPS = 1e-6 * N  # den eps, rescaled because kv/k_sum stay unscaled


def _split_multi_waits(nc, max_waits=1):
    """Walrus here allows one SyncWait per instruction; hoist extras onto
    fresh same-engine NOPs placed immediately before (same semantics)."""
    for f in nc.m.functions:
        for blk in f.blocks:
            new_insts = []
            for ins in blk.instructions:
                si = ins.sync_info
                waits = list(si.on_wait) if si is not None else []
                if len(waits) > max_waits:
                    head, tail = waits[:-max_waits], waits[-max_waits:]
                    for w in head:
                        nop = mybir.InstNoOp(
                            name=f"Wsplit-{nc.next_id()}", engine=ins.engine,
                            ins=[], outs=[],
                        )
                        nop.sync_info = mybir.SyncInfo(on_wait=[w], on_update=[])
                        new_insts.append(nop)
                    ins.sync_info = mybir.SyncInfo(
                        on_wait=tail, on_update=list(si.on_update)
                    )
                new_insts.append(ins)
            blk.instructions = new_insts


def _build():
    nc = bass.Bass()
    # all DRAM params are flat 1D: PJRT/XLA may permute multi-dim parameter
    # layouts (observed: [2,128,NP] stored as [128,2,NP]); 1D is unambiguous
    xs_f = nc.declare_dram_parameter("xs", [SPC * 2 * 128 * NP], F32R, isOutput=False)
    wqkT_f = nc.declare_dram_parameter("wqkT", [2 * 128 * 256], F32R, isOutput=False)
    wtap_f = nc.declare_dram_parameter("wtap", [2 * 9 * 128 * 128], F32R, isOutput=False)
    ident_f = nc.declare_dram_parameter("ident", [128 * 128], F32R, isOutput=False)
    ones_f = nc.declare_dram_parameter("ones", [128 * 128], F32R, isOutput=False)
    biasqk_f = nc.declare_dram_parameter("biasqk", [128 * 2], F32, isOutput=False)
    peb_f = nc.declare_dram_parameter("peb", [128 * 2], F32, isOutput=False)
    out_f = nc.declare_dram_parameter("out", [SPC * 2 * 128 * N], F32, isOutput=True)
    xs = xs_f[:].rearrange("(s c p n) -> s c p n", s=SPC, c=2, p=128)
    out = out_f[:].rearrange("(s c p n) -> s c p n", s=SPC, c=2, p=128)

    Exp = mybir.ActivationFunctionType.Exp
    Relu = mybir.ActivationFunctionType.Relu
    Ident = mybir.ActivationFunctionType.Identity
    mi, ad, mx = mybir.AluOpType.min, mybir.AluOpType.add, mybir.AluOpType.max

    def span_start(ch):
        # first output position of chunk ch, in padded coords
        return HP * (1 + 8 * ch) + 1

    with TileContext(nc) as tc:
        with (
            tc.tile_pool(name="wp", bufs=1) as wp,
            tc.tile_pool(name="xpool", bufs=2) as xpool,
            tc.tile_pool(name="qkpool", bufs=2) as qkpool,
            tc.tile_pool(name="erpool", bufs=2) as erpool,
            tc.tile_pool(name="xkpool", bufs=3) as xkpool,
            tc.tile_pool(name="kvpool", bufs=2) as kvpool,
            tc.tile_pool(name="denpool", bufs=1) as denpool,
            tc.tile_pool(name="opool", bufs=2) as opool,
            tc.tile_pool(name="bigps", bufs=3, space="PSUM") as bigps,
            tc.tile_pool(name="trps", bufs=2, space="PSUM") as trps,
            tc.tile_pool(name="kvps", bufs=2, space="PSUM") as kvps,
            tc.tile_pool(name="dbps", bufs=1, space="PSUM") as dbps,
        ):
            w_qk = wp.tile([128, 512], F32R, name="w_qk")
            w_tap = wp.tile([128, 2304], F32R, name="w_tap")
            w_id = wp.tile([128, 128], F32R, name="w_id")
            w_ones = wp.tile([128, 128], F32R, name="w_ones")
            w_bqk = wp.tile([128, 2], F32, name="w_bqk")
            w_peb = wp.tile([128, 2], F32, name="w_peb")
            dma = nc.default_dma_engine.dma_start
            dma(
                out=w_qk[:].rearrange("p (c o) -> p c o", c=2),
                in_=wqkT_f[:].rearrange("(c p o) -> p c o", c=2, p=128),
            )
            dma(
                out=w_tap[:].rearrange("p (c t j) -> p c t j", c=2, t=9),
                in_=wtap_f[:].rearrange("(c t p j) -> p c t j", c=2, t=9, p=128),
            )
            dma(out=w_id[:], in_=ident_f[:].rearrange("(p j) -> p j", p=128))
            dma(out=w_ones[:], in_=ones_f[:].rearrange("(p j) -> p j", p=128))
            dma(out=w_bqk[:], in_=biasqk_f[:].rearrange("(p c) -> p c", p=128))
            dma(out=w_peb[:], in_=peb_f[:].rearrange("(p c) -> p c", p=128))

            for s in range(SPC):
                # ---- A: load padded x -------------------------------------
                xp = xpool.tile([128, 2 * NP + 2], F32R, tag="xp", name="xp")
                for cb in range(2):
                    dma(out=xp[:, NP * cb : NP * (cb + 1)], in_=xs[s, cb])

                # ---- B: qk matmul + elu+1 ---------------------------------
                q_elu = qkpool.tile([128, NP], F32R, tag="qelu", name="q_elu")
                k_elu = qkpool.tile([128, NP], F32R, tag="kelu", name="k_elu")
                ksum7 = denpool.tile([128, 8], F32, tag="ksum7", name="ksum7")
                ksum = denpool.tile([128, 2], F32R, tag="ksum", name="ksum")
                # zero k's pad positions at tile birth (elu writes interior
                # only) so the kv/k_sum contraction over all 3364 padded
                # positions matches the dense reference exactly
                k_f32 = k_elu[:].bitcast(F32)
                nc.vector.memset(k_f32[:, 0:59], 0)
                nc.vector.memset(k_f32[:, NP - 58 : NP], 0)
                nc.vector.memset(
                    k_f32.rearrange("p (y x) -> p y x", y=HP)[:, 1:57, 0:1], 0
                )
                nc.vector.memset(
                    k_f32.rearrange("p (y x) -> p y x", y=HP)[:, 1:57, 57:58], 0
                )
                nc.vector.memset(ksum7[:].bitcast(F32), 0)
                for mb in range(2):  # 0 = q, 1 = k
                    dst = q_elu if mb == 0 else k_elu
                    for ch in range(NCH):
                        p1 = span_start(ch)
                        ps = bigps.tile([128, SPAN], F32, tag="bigps", name="ps")
                        for cc in range(2):
                            nc.tensor.matmul(
                                ps[:],
                                w_qk[:, 256 * cc + 128 * mb : 256 * cc + 128 * mb + 128],
                                xp[:, NP * cc + p1 : NP * cc + p1 + SPAN],
                                start=(cc == 0),
                                stop=(cc == 1),
                            )
                        e = erpool.tile([128, SPAN], F32, tag="e", name="e")
                        r = erpool.tile([128, SPAN], F32, tag="r", name="r")
                        nc.scalar.activation(
                            e[:], ps[:], Exp, bias=w_bqk[:, mb : mb + 1], scale=1.0
                        )
                        nc.vector.tensor_scalar(
                            out=r[:], in0=ps[:], scalar1=w_bqk[:, mb : mb + 1],
                            scalar2=0.0, op0=ad, op1=mx,
                        )
                        dst_v = dst[:, p1 : p1 + SPAN].rearrange(
                            "p (a b) -> p a b", b=HP
                        )[:, :, 0:56]
                        e_v = e[:].rearrange("p (a b) -> p a b", b=HP)[:, :, 0:56]
                        r_v = r[:].rearrange("p (a b) -> p a b", b=HP)[:, :, 0:56]
                        nc.vector.scalar_tensor_tensor(
                            dst_v, e_v, 1.0, r_v, op0=mi, op1=ad,
                            accum_out=(
                                ksum7[:, ch : ch + 1] if mb == 1 else None
                            ),
                        )
                with nc.allow_low_precision(
                    reason="ksum reduce to f32r: feeds f32r matmul anyway"
                ):
                    nc.vector.tensor_reduce(
                        ksum[:, 0:1], ksum7[:], op=mybir.AluOpType.add,
                        axis=mybir.AxisListType.X,
                    )

                # ---- C: transposes + kv -----------------------------------
                kvp = kvps.tile([128, 256], F32, tag="kvps", name="kvp")
                kv_sb = kvpool.tile([128, 256], F32R, tag="kv", name="kv_sb")
                for j in range(NTC):
                    tp = trps.tile([TCH, 384], F32R, tag="trps", name="tp")
                    for cb in range(2):
                        nc.tensor.transpose(
                            tp[:, 128 * cb : 128 * (cb + 1)],
                            xp[:, NP * cb + TCH * j : NP * cb + TCH * (j + 1)],
                            w_id[:],
                        )
                    nc.tensor.transpose(
                        tp[:, 256:384],
                        k_elu[:, TCH * j : TCH * (j + 1)],
                        w_id[:],
                    )
                    xk = xkpool.tile([TCH, 384], F32R, tag="xkt", name="xk")
                    if j % 2 == 0:
                        nc.scalar.copy(xk[:], tp[:])
                    else:
                        nc.vector.tensor_copy(xk[:], tp[:])
                    nc.tensor.matmul(
                        kvp[:, 0:256], xk[:, 256:384], xk[:, 0:256],
                        start=(j == 0), stop=(j == NTC - 1),
                    )
                nc.scalar.copy(kv_sb[:], kvp[:])

                # ---- D: den + reciprocal + q scaling ----------------------
                den = denpool.tile([1, NP], F32R, tag="den", name="den")
                for ch in range(NCH):
                    p1 = span_start(ch)
                    dp = dbps.tile([128, SPAN], F32, tag="dbps", name="dp")
                    nc.tensor.matmul(
                        dp[0:1, :], ksum[:, 0:1],
                        q_elu[:, p1 : p1 + SPAN],
                        start=True, stop=True,
                    )
                    nc.scalar.copy(den[:, p1 : p1 + SPAN], dp[0:1, :])
                recf = denpool.tile([TCH, NTC], F32R, tag="recf", name="recf")
                dma(out=recf[:], in_=den[:])
                nc.vector.tensor_scalar_add(recf[:], recf[:], EPS)
                with nc.allow_low_precision(
                    reason="f32r reciprocal: den ~O(n), 6e-5 rel is fine"
                ):
                    nc.vector.reciprocal(recf[:], recf[:])
                rrow = denpool.tile([1, NP], F32R, tag="rrow", name="rrow")
                dma(out=rrow[:], in_=recf[:])
                for ch in range(NCH):
                    p1 = span_start(ch)
                    bc = dbps.tile([128, SPAN], F32, tag="dbps", name="bc")
                    nc.tensor.matmul(
                        bc[:], w_ones[0:1, :], rrow[:, p1 : p1 + SPAN],
                        start=True, stop=True,
                    )
                    nc.vector.tensor_mul(
                        q_elu[:, p1 : p1 + SPAN], q_elu[:, p1 : p1 + SPAN], bc[:]
                    )

                # ---- E: num + conv taps into one psum; evac + bias --------
                for cb in range(2):
                    o_sb = opool.tile([128, N], F32, tag="osb", name="o_sb")
                    for ch in range(NCH):
                        p1 = span_start(ch)
                        pn = bigps.tile([128, SPAN], F32, tag="bigps", name="pn")
                        nc.tensor.matmul(
                            pn[:], kv_sb[:, 128 * cb : 128 * (cb + 1)],
                            q_elu[:, p1 : p1 + SPAN],
                            start=True, stop=False,
                        )
                        for t in range(9):
                            ky, kx = t // 3, t % 3
                            off = HP * (ky - 1) + (kx - 1)
                            nc.tensor.matmul(
                                pn[:],
                                w_tap[:, 1152 * cb + 128 * t : 1152 * cb + 128 * (t + 1)],
                                xp[:, NP * cb + p1 + off : NP * cb + p1 + off + SPAN],
                                start=False, stop=(t == 8),
                            )
                        nc.scalar.activation(
                            o_sb[:, 448 * ch : 448 * (ch + 1)].rearrange(
                                "p (y x) -> p y x", x=56
                            ),
                            pn[:].rearrange("p (y x) -> p y x", x=HP)[:, :, 0:56],
                            Ident, bias=w_peb[:, cb : cb + 1], scale=1.0,
                        )
                    dma(out=out[s, cb], in_=o_sb[:])

    _split_multi_waits(nc)
    return nc


_NC_CACHE = []


def kernel(x, qk_w, qk_b, pe_w, pe_b):
    x = np.asarray(x, np.float32)
    qk_w = np.asarray(qk_w, np.float32)
    qk_b = np.asarray(qk_b, np.float32)
    pe_w = np.asarray(pe_w, np.float32)
    pe_b = np.asarray(pe_b, np.float32)

    # host prep: zero-padded 58x58 spatial layout, c in two partition blocks
    xp = np.zeros((B, 2, 128, HP, HP), np.float32)
    xp[:, :, :, 1 : H + 1, 1 : W + 1] = x.reshape(B, 2, 128, H, W)
    xp = xp.reshape(B, 2, 128, NP)

    wqkT = np.ascontiguousarray(qk_w.T).reshape(2, 128, 256)
    wtap = np.zeros((2, 9, 128, 128), np.float32)
    idx = np.arange(128)
    for cb in range(2):
        for t in range(9):
            wtap[cb, t, idx, idx] = pe_w[128 * cb : 128 * (cb + 1), 0, t // 3, t % 3]
    biasqk = np.stack([qk_b[:128], qk_b[128:]], axis=1).copy()  # [128, 2]
    pebh = np.stack([pe_b[:128], pe_b[128:]], axis=1).copy()

    shared = {
        "wqkT": wqkT.ravel(),
        "wtap": wtap.ravel(),
        "ident": np.eye(128, dtype=np.float32).ravel(),
        "ones": np.ones(128 * 128, np.float32),
        "biasqk": biasqk.ravel(),
        "peb": pebh.ravel(),
    }
    in_maps = [
        {"xs": np.ascontiguousarray(xp[r * SPC : (r + 1) * SPC]).ravel(), **shared}
        for r in range(NCORES)
    ]

    if not _NC_CACHE:
        _NC_CACHE.append(_build())
    nc = _NC_CACHE[0]
    res = run_bass_kernel_spmd(nc, in_maps, list(range(NCORES)))

    full = np.empty((B, C, H, W), np.float32)
    for r in range(NCORES):
        o = res.results[r]["out"].reshape(SPC, 2, 128, N)
        full[r * SPC : (r + 1) * SPC] = o.reshape(SPC, C, H, W)
    return full



# revision 20
# speedup vs baseline: 1.2279x; 1.2279x over previous
"""Trainium2 Bass kernel for LinearAttention4 (self-contained).

Problem (per sample): x [256, 56, 56] fp32
  qk = elu(conv1x1(x; qk_w, qk_b)) + 1 ; q, k = split(qk)
  kv = k @ v.T ; num = q.T @ kv ; den = q.T @ sum(k) + eps ; attn = num/den
  out = attn + depthwise3x3(x; pe_w) + pe_b

Sharding: data-parallel over batch, 4 samples per core on 8 NeuronCores.

Dataflow (per sample, mostly bf16):
  A) DMA: x padded bf16 (qk rhs), x fp8 hi/lo padded (conv taps),
     xT bf16 transposed+padded (kv rhs) -- all host-prepped layout casts.
  B) qk matmul bf16 -> psum; elu+1 = min(exp(z+b), max(z+b+1, 1)) split as
     ACT Exp -> e, Pool tensor_scalar -> lin, DVE tensor_tensor min -> q/k.
  C) kT via one DMA-transpose instruction (bf16, 27x [128,128] blocks);
     ksum = kT.T @ ones and kv = kT.T @ xT accumulated on PE.
  D) den = ksum.T @ q per span; fold [1,NP]->[116,29] via DMA, +eps,
     reciprocal -> bf16, unfold; bc = ones-row broadcast matmul; q *= bc.
  E) per (c-block, span): 14 fp8 DoubleRow tap matmuls (w split hi+lo on
     x8hi, plus w-hi on x8lo: exact to ~2.6e-3 of scale) + bf16 num matmul
     into one psum tile; ACT evacuates + pe_b, compacting to dense 56x56;
     bf16 out DMA, host casts to f32.
"""

import numpy as np
import ml_dtypes

import bass_rust
import concourse.bass as bass
import concourse.mybir as mybir
from concourse.tile import TileContext
from concourse.bass_utils import run_bass_kernel_spmd

F32 = mybir.dt.float32
BF16 = mybir.dt.bfloat16
FP8 = mybir.dt.float8e4

B, C, H, W = 32, 256, 56, 56
N = H * W  # 3136
NCORES = 8
SPC = B // NCORES  # 4
HP = H + 2  # 58
NP = HP * HP  # 3364
NPP = 27 * 128  # 3456: NP padded so DMA-transpose tiles divide evenly
SPAN = 8 * HP  # 464 cols per span (8 padded rows)
NCH = 7  # spans of 8 interior rows
EPS = 1e-6 * N  # den eps, rescaled because kv/ksum stay unscaled

# conv tap byte offsets into the padded grid, t = 3*dy + dx
OFFS = [HP * (dy - 1) + (dx - 1) for dy in range(3) for dx in range(3)]
# DR-b pairs for the w_hi * x_lo products (last tap pairs with a zero slot)
BPAIRS = [(0, 1), (2, 3), (4, 5), (6, 7), (8, None)]


def _split_multi_waits(nc, max_waits=1):
    """Walrus allows one SyncWait per instruction; hoist extras onto
    fresh same-engine NOPs placed immediately before (same semantics)."""
    for f in nc.m.functions:
        for blk in f.blocks:
            new_insts = []
            for ins in blk.instructions:
                si = ins.sync_info
                waits = list(si.on_wait) if si is not None else []
                if len(waits) > max_waits:
                    head, tail = waits[:-max_waits], waits[-max_waits:]
                    for w in head:
                        nop = mybir.InstNoOp(
                            name=f"Wsplit-{nc.next_id()}", engine=ins.engine,
                            ins=[], outs=[],
                        )
                        nop.sync_info = mybir.SyncInfo(on_wait=[w], on_update=[])
                        new_insts.append(nop)
                    ins.sync_info = mybir.SyncInfo(
                        on_wait=tail, on_update=list(si.on_update)
                    )
                new_insts.append(ins)
            blk.instructions = new_insts


def _pair_ap(tile_ap, base, delta, width=SPAN):
    """rhs view [128, 2, width] over a padded fp8 buffer: slot j reads at
    byte offset base + j*delta (delta may be 0 to read the same span twice)."""
    ap = tile_ap[:, base : base + width]
    ap2 = ap.copy()
    ap2.ap = bass_rust.VecI64Pair(
        [list(ap.ap[0]), [delta, 2], [1, width]]
    )
    return ap2


def _build(phases="ABMQCDETN"):
    nc = bass.Bass()
    # flat 1D DRAM params: PJRT/XLA may permute multi-dim parameter layouts
    xb_f = nc.declare_dram_parameter("xb", [SPC * 2 * 128 * NP], BF16, isOutput=False)
    xT_f = nc.declare_dram_parameter("xT", [SPC * 27 * 128 * 256], BF16, isOutput=False)
    wqk_f = nc.declare_dram_parameter("wqk", [128 * 2 * 2 * 128], BF16, isOutput=False)
    wtap_f = nc.declare_dram_parameter("wtap", [2 * 9 * 128 * 128], BF16, isOutput=False)
    ones_f = nc.declare_dram_parameter("ones", [128 * 128], BF16, isOutput=False)
    bqk_f = nc.declare_dram_parameter("bqk", [128 * 2], F32, isOutput=False)
    bq1_f = nc.declare_dram_parameter("bq1", [128 * 2], F32, isOutput=False)
    peb_f = nc.declare_dram_parameter("peb", [128 * 2], F32, isOutput=False)
    out_f = nc.declare_dram_parameter("out", [SPC * 2 * 128 * N], BF16, isOutput=True)

    xbs = xb_f[:].rearrange("(s c p n) -> s p c n", s=SPC, c=2, p=128)
    xTs = xT_f[:].rearrange("(s j p c) -> s p j c", s=SPC, j=27, p=128)
    out = out_f[:].rearrange("(s c p n) -> s c p n", s=SPC, c=2, p=128)

    Exp = mybir.ActivationFunctionType.Exp
    Ident = mybir.ActivationFunctionType.Identity
    ad, mx, mi, mu = (mybir.AluOpType.add, mybir.AluOpType.max,
                      mybir.AluOpType.min, mybir.AluOpType.mult)

    def span_start(ch):
        return HP * (1 + 8 * ch) + 1

    with TileContext(nc) as tc:
        with (
            tc.tile_pool(name="wp", bufs=1) as wp,
            tc.tile_pool(name="xpool", bufs=2) as xpool,
            tc.tile_pool(name="qkpool", bufs=2) as qkpool,
            tc.tile_pool(name="erpool", bufs=3) as erpool,
            tc.tile_pool(name="ktpool", bufs=2) as ktpool,
            tc.tile_pool(name="kvpool", bufs=2) as kvpool,
            tc.tile_pool(name="denpool", bufs=1) as denpool,
            tc.tile_pool(name="opool", bufs=2) as opool,
            tc.tile_pool(name="qkps", bufs=2, space="PSUM") as qkps,
            tc.tile_pool(name="kvps", bufs=1, space="PSUM") as kvps,
            tc.tile_pool(name="ksps", bufs=1, space="PSUM") as ksps,
            tc.tile_pool(name="dbps", bufs=2, space="PSUM") as dbps,
            tc.tile_pool(name="bigps", bufs=2, space="PSUM") as bigps,
        ):
            w_qk = wp.tile([128, 512], BF16, name="w_qk")
            w_tap = wp.tile([128, 2 * 9 * 128], BF16, name="w_tap")
            w_ones = wp.tile([128, 128], BF16, name="w_ones")
            w_bqk = wp.tile([128, 2], F32, name="w_bqk")
            w_bq1 = wp.tile([128, 2], F32, name="w_bq1")
            w_peb = wp.tile([128, 2], F32, name="w_peb")
            dma = nc.default_dma_engine.dma_start
            dma(out=w_qk[:], in_=wqk_f[:].rearrange("(p q) -> p q", p=128))
            dma(
                out=w_tap[:].rearrange("p (c k m) -> p c k m", c=2, k=9),
                in_=wtap_f[:].rearrange("(c k p m) -> p c k m", c=2, k=9, p=128),
            )
            dma(out=w_ones[:], in_=ones_f[:].rearrange("(p j) -> p j", p=128))
            dma(out=w_bqk[:], in_=bqk_f[:].rearrange("(p c) -> p c", p=128))
            dma(out=w_bq1[:], in_=bq1_f[:].rearrange("(p c) -> p c", p=128))
            dma(out=w_peb[:], in_=peb_f[:].rearrange("(p c) -> p c", p=128))
            wtv = w_tap[:].rearrange("p (c k m) -> p c k m", c=2, k=9)

            for s in range(SPC):
                # ---- A: loads ---------------------------------------------
                xbt = xpool.tile([128, 2 * NP + 2], BF16, tag="xb", name="xbt")
                xTt = xpool.tile([128, 27 * 256], BF16, tag="xT", name="xTt")
                dma(out=xbt[:, 0 : 2 * NP].rearrange("p (c n) -> p c n", c=2), in_=xbs[s])
                dma(out=xTt[:].rearrange("p (j c) -> p j c", j=27), in_=xTs[s])
                nc.vector.memset(xbt[:, 2 * NP : 2 * NP + 2], 0)

                if "B" not in phases:
                    continue
                # ---- B: qk matmul + elu+1 ---------------------------------
                q_t = qkpool.tile([128, NP], BF16, tag="q", name="q_t")
                k_t = qkpool.tile([128, NPP], BF16, tag="k", name="k_t")
                # zero grid pads (and k's transpose tail) so kv/ksum/den see
                # exact zeros there
                if "M" in phases:
                    for t in (q_t, k_t):
                        nc.vector.memset(t[:, 0:59], 0)
                        pairs = t[:, 115 : 115 + 58 * 55 + 2]
                        pap = pairs.copy()
                        pap.ap = bass_rust.VecI64Pair(
                            [list(pairs.ap[0]), [58, 56], [1, 2]]
                        )
                        nc.vector.memset(pap, 0)
                        nc.vector.memset(
                            t[:, 3307 : (3307 + 57 if t is q_t else NPP)], 0
                        )
                if "B" not in phases or "Q" not in phases:
                    continue
                for mb in range(2):  # 0 = q, 1 = k
                    dst = q_t if mb == 0 else k_t
                    for ch in range(NCH):
                        p1 = span_start(ch)
                        ps = qkps.tile([128, SPAN], F32, tag="qkps", name="ps")
                        for cc in range(2):
                            nc.tensor.matmul(
                                ps[:],
                                w_qk[:].rearrange(
                                    "p (cc mb o) -> p cc mb o", cc=2, mb=2
                                )[:, cc, mb],
                                xbt[:, NP * cc + p1 : NP * cc + p1 + SPAN],
                                start=(cc == 0),
                                stop=(cc == 1),
                            )
                        e = erpool.tile([128, SPAN], BF16, tag="e", name="e")
                        lin = erpool.tile([128, SPAN], BF16, tag="lin", name="lin")
                        nc.scalar.activation(
                            e[:], ps[:], Exp, bias=w_bqk[:, mb : mb + 1], scale=1.0
                        )
                        nc.vector.tensor_scalar(
                            out=lin[:], in0=ps[:], scalar1=w_bq1[:, mb : mb + 1],
                            scalar2=1.0, op0=ad, op1=mx,
                        )
                        dst_v = dst[:, p1 : p1 + SPAN].rearrange(
                            "p (a b) -> p a b", b=HP
                        )[:, :, 0:56]
                        e_v = e[:].rearrange("p (a b) -> p a b", b=HP)[:, :, 0:56]
                        l_v = lin[:].rearrange("p (a b) -> p a b", b=HP)[:, :, 0:56]
                        nc.vector.tensor_tensor(dst_v, e_v, l_v, op=mi)

                if "C" not in phases:
                    continue
                # ---- C: kT transpose + ksum + kv --------------------------
                kT = ktpool.tile([128, 27 * 128], BF16, tag="kT", name="kT")
                nc.default_dma_engine.dma_start_transpose(
                    kT[:].rearrange("p (j d) -> p j d", j=27), k_t[:]
                )
                ksp = ksps.tile([128, 2], F32, tag="ksp", name="ksp")
                kvp = kvps.tile([128, 256], F32, tag="kvp", name="kvp")
                for j in range(27):
                    nc.tensor.matmul(
                        ksp[:, 0:1], kT[:, 128 * j : 128 * (j + 1)],
                        w_ones[:, 0:1],
                        start=(j == 0), stop=(j == 26),
                    )
                for j in range(27):
                    nc.tensor.matmul(
                        kvp[:], kT[:, 128 * j : 128 * (j + 1)],
                        xTt[:, 256 * j : 256 * (j + 1)],
                        start=(j == 0), stop=(j == 26),
                    )
                ksumb = denpool.tile([128, 2], BF16, tag="ksumb", name="ksumb")
                kvb = kvpool.tile([128, 256], BF16, tag="kvb", name="kvb")
                nc.scalar.copy(ksumb[:, 0:1], ksp[:, 0:1])
                nc.scalar.copy(kvb[:], kvp[:])

                if "D" not in phases:
                    continue
                # ---- D: den + reciprocal + q scaling ----------------------
                den = denpool.tile([1, NP], F32, tag="den", name="den")
                for ch in range(NCH):
                    p1 = span_start(ch)
                    dp = dbps.tile([128, SPAN], F32, tag="dbps", name="dp")
                    nc.tensor.matmul(
                        dp[0:1, :], ksumb[:, 0:1],
                        q_t[:, p1 : p1 + SPAN],
                        start=True, stop=True,
                    )
                    nc.scalar.copy(den[:, p1 : p1 + SPAN], dp[0:1, :])
                recf = denpool.tile([116, 29], F32, tag="recf", name="recf")
                recb = denpool.tile([116, 29], BF16, tag="recb", name="recb")
                dma(out=recf[:], in_=den[:])
                nc.vector.tensor_scalar_add(recf[:], recf[:], EPS)
                with nc.allow_low_precision(
                    reason="bf16 reciprocal of den ~O(n): ~0.4% rel, fine"
                ):
                    nc.vector.reciprocal(recb[:], recf[:])
                rrow = denpool.tile([1, NP], BF16, tag="rrow", name="rrow")
                dma(out=rrow[:], in_=recb[:])
                for ch in range(NCH):
                    p1 = span_start(ch)
                    bc = dbps.tile([128, SPAN], F32, tag="dbps", name="bc")
                    nc.tensor.matmul(
                        bc[:], w_ones[0:1, :], rrow[:, p1 : p1 + SPAN],
                        start=True, stop=True,
                    )
                    nc.vector.tensor_tensor(
                        q_t[:, p1 : p1 + SPAN], q_t[:, p1 : p1 + SPAN], bc[:],
                        op=mu,
                    )

                if "E" not in phases:
                    continue
                # ---- E: taps (fp8 DoubleRow) + num into one psum ----------
                for cb in range(2):
                    o_sb = opool.tile([128, N], BF16, tag="osb", name="o_sb")
                    for ch in range(NCH):
                        p1 = span_start(ch)
                        pn = bigps.tile([128, SPAN], F32, tag="bigps", name="pn")
                        base = NP * cb + p1
                        full = not any(f in phases for f in "TN")
                        ops = []
                        if full or "T" in phases:  # bf16 diagonal conv taps
                            for t in range(9):
                                ops.append((wtv[:, cb, t],
                                            xbt[:, base + OFFS[t] : base + OFFS[t] + SPAN]))
                        if full or "N" in phases:
                            ops.append((kvb[:, 128 * cb : 128 * (cb + 1)],
                                        q_t[:, p1 : p1 + SPAN]))
                        for i, (lhs, rhs) in enumerate(ops):
                            nc.tensor.matmul(
                                pn[:], lhs, rhs,
                                start=(i == 0), stop=(i == len(ops) - 1),
                                skip_group_check=True,
                            )
                        nc.scalar.activation(
                            o_sb[:, 448 * ch : 448 * (ch + 1)].rearrange(
                                "p (y x) -> p y x", x=56
                            ),
                            pn[:].rearrange("p (y x) -> p y x", x=HP)[:, :, 0:56],
                            Ident, bias=w_peb[:, cb : cb + 1], scale=1.0,
                        )
                    dma(out=out[s, cb], in_=o_sb[:])

    _split_multi_waits(nc)
    return nc


_NC_CACHE = []


def kernel(x, qk_w, qk_b, pe_w, pe_b):
    x = np.asarray(x, np.float32)
    qk_w = np.asarray(qk_w, np.float32)
    qk_b = np.asarray(qk_b, np.float32)
    pe_w = np.asarray(pe_w, np.float32)
    pe_b = np.asarray(pe_b, np.float32)

    # host prep: zero-padded 58x58 layout + dtype casts (no arithmetic)
    xpad = np.zeros((B, 2, 128, HP, HP), np.float32)
    xpad[:, :, :, 1 : H + 1, 1 : W + 1] = x.reshape(B, 2, 128, H, W)
    xpad = xpad.reshape(B, 2, 128, NP)
    xb16 = xpad.astype(ml_dtypes.bfloat16)
    xT = np.zeros((B, NPP, 256), np.float32)
    xT[:, :NP] = xpad.reshape(B, 256, NP).transpose(0, 2, 1)
    xT16 = xT.reshape(B, 27, 128, 256).astype(ml_dtypes.bfloat16)

    wqk = np.ascontiguousarray(
        qk_w.T.reshape(2, 128, 2, 128).transpose(1, 0, 2, 3)
    ).astype(ml_dtypes.bfloat16)  # [p, cc, mb, o]

    w0 = pe_w[:, 0].reshape(256, 9)
    wtap = np.zeros((2, 9, 128, 128), ml_dtypes.bfloat16)
    idx = np.arange(128)
    for cb in range(2):
        sl = slice(128 * cb, 128 * (cb + 1))
        for t in range(9):
            wtap[cb, t, idx, idx] = w0[sl, t]

    bqk = np.stack([qk_b[:128], qk_b[128:]], axis=1).astype(np.float32)
    bq1 = bqk + 1.0
    pebh = np.stack([pe_b[:128], pe_b[128:]], axis=1).astype(np.float32)

    shared = {
        "wqk": wqk.ravel(),
        "wtap": wtap.ravel(),
        "ones": np.ones(128 * 128, ml_dtypes.bfloat16),
        "bqk": np.ascontiguousarray(bqk).ravel(),
        "bq1": np.ascontiguousarray(bq1).ravel(),
        "peb": np.ascontiguousarray(pebh).ravel(),
    }
    in_maps = [
        {
            "xb": np.ascontiguousarray(xb16[r * SPC : (r + 1) * SPC]).ravel(),
            "xT": np.ascontiguousarray(xT16[r * SPC : (r + 1) * SPC]).ravel(),
            **shared,
        }
        for r in range(NCORES)
    ]

    if not _NC_CACHE:
        _NC_CACHE.append(_build())
    nc = _NC_CACHE[0]
    res = run_bass_kernel_spmd(nc, in_maps, list(range(NCORES)))

    full = np.empty((B, C, H, W), np.float32)
    for r in range(NCORES):
        o = res.results[r]["out"].astype(np.float32).reshape(SPC, 2, 128, N)
        full[r * SPC : (r + 1) * SPC] = o.reshape(SPC, C, H, W)
    return full


# revision 23
# speedup vs baseline: 1.3709x; 1.1165x over previous
"""Trainium2 Bass kernel for LinearAttention4 (self-contained).

Problem (per sample): x [256, 56, 56] fp32
  qk = elu(conv1x1(x; qk_w, qk_b)) + 1 ; q, k = split(qk)
  kv = k @ v.T ; num = q.T @ kv ; den = q.T @ sum(k) + eps ; attn = num/den
  out = attn + depthwise3x3(x; pe_w) + pe_b

Sharding: data-parallel over batch, 4 samples per core on 8 NeuronCores.

Dataflow (per sample, mostly bf16):
  A) DMA: x padded bf16 (qk rhs + conv taps), xT bf16 transposed+padded
     (kv rhs) -- all host-prepped layout/dtype casts, no host arithmetic.
  B) qk matmul bf16 -> psum; elu+1 = min(exp(z+b), max(z+b+1, 1)) split as
     ACT Exp -> e, DVE tensor_scalar -> lin, DVE tensor_tensor min -> q/k.
  C) kT via one DMA-transpose instruction (bf16, 27x [128,128] blocks);
     ksum = kT.T @ ones and kv = kT.T @ xT accumulated on PE.
  D) den = ksum.T @ q per span; fold [1,NP]->[116,29] via DMA, +eps,
     reciprocal -> bf16, unfold; bc = ones-row broadcast matmul; q *= bc.
  E) per (c-block, span): 9 bf16 diagonal tap matmuls (exact depthwise
     conv) + bf16 num matmul into one psum tile; ACT evacuates + pe_b,
     compacting to dense 56x56; bf16 out DMA, host casts to f32.
  Sample pairs are software-pipelined so PE always has the sibling
  sample's matmuls during the kT-transpose and reciprocal waits.
"""

import numpy as np
import ml_dtypes

import bass_rust
import concourse.bass as bass
import concourse.mybir as mybir
from concourse.tile import TileContext
from concourse.bass_utils import run_bass_kernel_spmd

F32 = mybir.dt.float32
BF16 = mybir.dt.bfloat16
FP8 = mybir.dt.float8e4

B, C, H, W = 32, 256, 56, 56
N = H * W  # 3136
NCORES = 8
SPC = B // NCORES  # 4
HP = H + 2  # 58
NP = HP * HP  # 3364
NPP = 27 * 128  # 3456: NP padded so DMA-transpose tiles divide evenly
SPAN = 8 * HP  # 464 cols per span (8 padded rows)
NCH = 7  # spans of 8 interior rows
EPS = 1e-6 * N  # den eps, rescaled because kv/ksum stay unscaled

# conv tap byte offsets into the padded grid, t = 3*dy + dx
OFFS = [HP * (dy - 1) + (dx - 1) for dy in range(3) for dx in range(3)]


def _split_multi_waits(nc, max_waits=1):
    """Walrus allows one SyncWait per instruction; hoist extras onto
    fresh same-engine NOPs placed immediately before (same semantics)."""
    for f in nc.m.functions:
        for blk in f.blocks:
            new_insts = []
            for ins in blk.instructions:
                si = ins.sync_info
                waits = list(si.on_wait) if si is not None else []
                if len(waits) > max_waits:
                    head, tail = waits[:-max_waits], waits[-max_waits:]
                    for w in head:
                        nop = mybir.InstNoOp(
                            name=f"Wsplit-{nc.next_id()}", engine=ins.engine,
                            ins=[], outs=[],
                        )
                        nop.sync_info = mybir.SyncInfo(on_wait=[w], on_update=[])
                        new_insts.append(nop)
                    ins.sync_info = mybir.SyncInfo(
                        on_wait=tail, on_update=list(si.on_update)
                    )
                new_insts.append(ins)
            blk.instructions = new_insts


def _pair_ap(tile_ap, base, delta, width=SPAN):
    """rhs view [128, 2, width] over a padded fp8 buffer: slot j reads at
    byte offset base + j*delta (delta may be 0 to read the same span twice)."""
    ap = tile_ap[:, base : base + width]
    ap2 = ap.copy()
    ap2.ap = bass_rust.VecI64Pair(
        [list(ap.ap[0]), [delta, 2], [1, width]]
    )
    return ap2


def _build(phases="ABMQCDETN"):
    nc = bass.Bass()
    # flat 1D DRAM params: PJRT/XLA may permute multi-dim parameter layouts
    xb_f = nc.declare_dram_parameter("xb", [SPC * 2 * 128 * NP], BF16, isOutput=False)
    xT_f = nc.declare_dram_parameter("xT", [SPC * 27 * 128 * 256], BF16, isOutput=False)
    wqk_f = nc.declare_dram_parameter("wqk", [128 * 2 * 2 * 128], BF16, isOutput=False)
    wtap_f = nc.declare_dram_parameter("wtap", [2 * 9 * 128 * 128], BF16, isOutput=False)
    ones_f = nc.declare_dram_parameter("ones", [128 * 128], BF16, isOutput=False)
    bqk_f = nc.declare_dram_parameter("bqk", [128 * 2], F32, isOutput=False)
    bq1_f = nc.declare_dram_parameter("bq1", [128 * 2], F32, isOutput=False)
    peb_f = nc.declare_dram_parameter("peb", [128 * 2], F32, isOutput=False)
    out_f = nc.declare_dram_parameter("out", [SPC * 2 * 128 * N], BF16, isOutput=True)

    xbs = xb_f[:].rearrange("(s c p n) -> s p c n", s=SPC, c=2, p=128)
    xTs = xT_f[:].rearrange("(s j p c) -> s p j c", s=SPC, j=27, p=128)
    out = out_f[:].rearrange("(s c p n) -> s c p n", s=SPC, c=2, p=128)

    Exp = mybir.ActivationFunctionType.Exp
    Ident = mybir.ActivationFunctionType.Identity
    ad, mx, mi, mu = (mybir.AluOpType.add, mybir.AluOpType.max,
                      mybir.AluOpType.min, mybir.AluOpType.mult)

    def span_start(ch):
        return HP * (1 + 8 * ch) + 1

    with TileContext(nc) as tc:
        with (
            tc.tile_pool(name="wp", bufs=1) as wp,
            tc.tile_pool(name="xpool", bufs=3) as xpool,
            tc.tile_pool(name="qkpool", bufs=2) as qkpool,
            tc.tile_pool(name="erpool", bufs=4) as erpool,
            tc.tile_pool(name="ktpool", bufs=2) as ktpool,
            tc.tile_pool(name="kvpool", bufs=2) as kvpool,
            tc.tile_pool(name="denpool", bufs=1) as denpool,
            tc.tile_pool(name="opool", bufs=2) as opool,
            tc.tile_pool(name="qkps", bufs=3, space="PSUM") as qkps,
            tc.tile_pool(name="kvps", bufs=1, space="PSUM") as kvps,
            tc.tile_pool(name="dbps", bufs=2, space="PSUM") as dbps,
            tc.tile_pool(name="bigps", bufs=2, space="PSUM") as bigps,
        ):
            w_qk = wp.tile([128, 512], BF16, name="w_qk")
            w_tap = wp.tile([128, 2 * 9 * 128], BF16, name="w_tap")
            w_ones = wp.tile([128, 128], BF16, name="w_ones")
            w_bqk = wp.tile([128, 2], F32, name="w_bqk")
            w_bq1 = wp.tile([128, 2], F32, name="w_bq1")
            w_peb = wp.tile([128, 2], F32, name="w_peb")
            dma = nc.default_dma_engine.dma_start
            dma(out=w_qk[:], in_=wqk_f[:].rearrange("(p q) -> p q", p=128))
            dma(
                out=w_tap[:].rearrange("p (c k m) -> p c k m", c=2, k=9),
                in_=wtap_f[:].rearrange("(c k p m) -> p c k m", c=2, k=9, p=128),
            )
            dma(out=w_ones[:], in_=ones_f[:].rearrange("(p j) -> p j", p=128))
            dma(out=w_bqk[:], in_=bqk_f[:].rearrange("(p c) -> p c", p=128))
            dma(out=w_bq1[:], in_=bq1_f[:].rearrange("(p c) -> p c", p=128))
            dma(out=w_peb[:], in_=peb_f[:].rearrange("(p c) -> p c", p=128))
            wtv = w_tap[:].rearrange("p (c k m) -> p c k m", c=2, k=9)

            def phase_A(s):
                xbt = xpool.tile([128, 2 * NP + 2], BF16, tag="xb", name="xbt")
                xTt = xpool.tile([128, 27 * 256], BF16, tag="xT", name="xTt")
                dma(out=xbt[:, 0 : 2 * NP].rearrange("p (c n) -> p c n", c=2), in_=xbs[s])
                dma(out=xTt[:].rearrange("p (j c) -> p j c", j=27), in_=xTs[s])
                nc.vector.memset(xbt[:, 2 * NP : 2 * NP + 2], 0)
                return {"xbt": xbt, "xTt": xTt}

            def phase_B(s, st):
                xbt = st["xbt"]
                q_t = qkpool.tile([128, NP], BF16, tag="q", name="q_t")
                k_t = qkpool.tile([128, NPP], BF16, tag="k", name="k_t")
                for t in (q_t, k_t):
                    nc.vector.memset(t[:, 0:59], 0)
                    pairs = t[:, 115 : 115 + 58 * 55 + 2]
                    pap = pairs.copy()
                    pap.ap = bass_rust.VecI64Pair(
                        [list(pairs.ap[0]), [58, 56], [1, 2]]
                    )
                    nc.vector.memset(pap, 0)
                    nc.vector.memset(
                        t[:, 3307 : (3307 + 57 if t is q_t else NPP)], 0
                    )
                for mb in range(2):  # 0 = q, 1 = k
                    dst = q_t if mb == 0 else k_t
                    for ch in range(NCH):
                        p1 = span_start(ch)
                        ps = qkps.tile([128, SPAN], F32, tag="qkps", name="ps")
                        for cc in range(2):
                            nc.tensor.matmul(
                                ps[:],
                                w_qk[:].rearrange(
                                    "p (cc mb o) -> p cc mb o", cc=2, mb=2
                                )[:, cc, mb],
                                xbt[:, NP * cc + p1 : NP * cc + p1 + SPAN],
                                start=(cc == 0),
                                stop=(cc == 1),
                            )
                        e = erpool.tile([128, SPAN], BF16, tag="e", name="e")
                        lin = erpool.tile([128, SPAN], BF16, tag="lin", name="lin")
                        nc.scalar.activation(
                            e[:], ps[:], Exp, bias=w_bqk[:, mb : mb + 1], scale=1.0
                        )
                        nc.vector.tensor_scalar(
                            out=lin[:], in0=ps[:], scalar1=w_bq1[:, mb : mb + 1],
                            scalar2=1.0, op0=ad, op1=mx,
                        )
                        dst_v = dst[:, p1 : p1 + SPAN].rearrange(
                            "p (a b) -> p a b", b=HP
                        )[:, :, 0:56]
                        e_v = e[:].rearrange("p (a b) -> p a b", b=HP)[:, :, 0:56]
                        l_v = lin[:].rearrange("p (a b) -> p a b", b=HP)[:, :, 0:56]
                        nc.vector.tensor_tensor(dst_v, e_v, l_v, op=mi)
                st.update({"q_t": q_t, "k_t": k_t})

            def phase_C(s, st):
                k_t, xTt = st["k_t"], st["xTt"]
                kT = ktpool.tile([128, 27 * 128], BF16, tag="kT", name="kT")
                nc.default_dma_engine.dma_start_transpose(
                    kT[:].rearrange("p (j d) -> p j d", j=27), k_t[:]
                )
                kvp = kvps.tile([128, 258], F32, tag="kvp", name="kvp")
                for j in range(27):
                    nc.tensor.matmul(
                        kvp[:, 256:257], kT[:, 128 * j : 128 * (j + 1)],
                        w_ones[:, 0:1],
                        start=(j == 0), stop=(j == 26),
                        skip_group_check=True,
                    )
                for j in range(27):
                    nc.tensor.matmul(
                        kvp[:, 0:256], kT[:, 128 * j : 128 * (j + 1)],
                        xTt[:, 256 * j : 256 * (j + 1)],
                        start=(j == 0), stop=(j == 26),
                        skip_group_check=True,
                    )
                ksumb = denpool.tile([128, 2], BF16, tag="ksumb", name="ksumb")
                kvb = kvpool.tile([128, 256], BF16, tag="kvb", name="kvb")
                nc.scalar.copy(ksumb[:, 0:1], kvp[:, 256:257])
                nc.scalar.copy(kvb[:], kvp[:, 0:256])
                st.update({"ksumb": ksumb, "kvb": kvb})

            def phase_D(s, st):
                q_t, ksumb = st["q_t"], st["ksumb"]
                den = denpool.tile([1, NP], F32, tag="den", name="den")
                for ch in range(NCH):
                    p1 = span_start(ch)
                    dp = dbps.tile([128, SPAN], F32, tag="dbps", name="dp")
                    nc.tensor.matmul(
                        dp[0:1, :], ksumb[:, 0:1],
                        q_t[:, p1 : p1 + SPAN],
                        start=True, stop=True,
                    )
                    nc.scalar.copy(den[:, p1 : p1 + SPAN], dp[0:1, :])
                recf = denpool.tile([116, 29], F32, tag="recf", name="recf")
                recb = denpool.tile([116, 29], BF16, tag="recb", name="recb")
                dma(out=recf[:], in_=den[:])
                nc.vector.tensor_scalar_add(recf[:], recf[:], EPS)
                with nc.allow_low_precision(
                    reason="bf16 reciprocal of den ~O(n): ~0.4% rel, fine"
                ):
                    nc.vector.reciprocal(recb[:], recf[:])
                rrow = denpool.tile([1, NP], BF16, tag="rrow", name="rrow")
                dma(out=rrow[:], in_=recb[:])
                for ch in range(NCH):
                    p1 = span_start(ch)
                    bc = dbps.tile([128, SPAN], F32, tag="dbps", name="bc")
                    nc.tensor.matmul(
                        bc[:], w_ones[0:1, :], rrow[:, p1 : p1 + SPAN],
                        start=True, stop=True,
                    )
                    nc.vector.tensor_tensor(
                        q_t[:, p1 : p1 + SPAN], q_t[:, p1 : p1 + SPAN], bc[:],
                        op=mu,
                    )

            def phase_E(s, st):
                xbt, q_t, kvb = st["xbt"], st["q_t"], st["kvb"]
                for cb in range(2):
                    o_sb = opool.tile([128, N], BF16, tag="osb", name="o_sb")
                    for ch in range(NCH):
                        p1 = span_start(ch)
                        pn = bigps.tile([128, SPAN], F32, tag="bigps", name="pn")
                        base = NP * cb + p1
                        for t in range(9):  # bf16 diagonal conv taps
                            nc.tensor.matmul(
                                pn[:], wtv[:, cb, t],
                                xbt[:, base + OFFS[t] : base + OFFS[t] + SPAN],
                                start=(t == 0), stop=False,
                                skip_group_check=True,
                            )
                        nc.tensor.matmul(
                            pn[:], kvb[:, 128 * cb : 128 * (cb + 1)],
                            q_t[:, p1 : p1 + SPAN],
                            start=False, stop=True,
                            skip_group_check=True,
                        )
                        nc.scalar.activation(
                            o_sb[:, 448 * ch : 448 * (ch + 1)].rearrange(
                                "p (y x) -> p y x", x=56
                            ),
                            pn[:].rearrange("p (y x) -> p y x", x=HP)[:, :, 0:56],
                            Ident, bias=w_peb[:, cb : cb + 1], scale=1.0,
                        )
                    dma(out=out[s, cb], in_=o_sb[:])

            # software pipeline: interleave sample pairs so PE always has the
            # sibling sample's matmuls during kT-transpose / reciprocal waits
            for s0 in range(0, SPC, 2):
                s1 = s0 + 1
                stA = phase_A(s0)
                stB = phase_A(s1)
                phase_B(s0, stA)
                phase_B(s1, stB)
                phase_C(s0, stA)
                phase_C(s1, stB)
                phase_D(s0, stA)
                phase_D(s1, stB)
                phase_E(s0, stA)
                phase_E(s1, stB)

    _split_multi_waits(nc)
    return nc


_NC_CACHE = []


def kernel(x, qk_w, qk_b, pe_w, pe_b):
    x = np.asarray(x, np.float32)
    qk_w = np.asarray(qk_w, np.float32)
    qk_b = np.asarray(qk_b, np.float32)
    pe_w = np.asarray(pe_w, np.float32)
    pe_b = np.asarray(pe_b, np.float32)

    # host prep: zero-padded 58x58 layout + dtype casts (no arithmetic)
    xpad = np.zeros((B, 2, 128, HP, HP), np.float32)
    xpad[:, :, :, 1 : H + 1, 1 : W + 1] = x.reshape(B, 2, 128, H, W)
    xpad = xpad.reshape(B, 2, 128, NP)
    xb16 = xpad.astype(ml_dtypes.bfloat16)
    xT = np.zeros((B, NPP, 256), np.float32)
    xT[:, :NP] = xpad.reshape(B, 256, NP).transpose(0, 2, 1)
    xT16 = xT.reshape(B, 27, 128, 256).astype(ml_dtypes.bfloat16)

    wqk = np.ascontiguousarray(
        qk_w.T.reshape(2, 128, 2, 128).transpose(1, 0, 2, 3)
    ).astype(ml_dtypes.bfloat16)  # [p, cc, mb, o]

    w0 = pe_w[:, 0].reshape(256, 9)
    wtap = np.zeros((2, 9, 128, 128), ml_dtypes.bfloat16)
    idx = np.arange(128)
    for cb in range(2):
        sl = slice(128 * cb, 128 * (cb + 1))
        for t in range(9):
            wtap[cb, t, idx, idx] = w0[sl, t]

    bqk = np.stack([qk_b[:128], qk_b[128:]], axis=1).astype(np.float32)
    bq1 = bqk + 1.0
    pebh = np.stack([pe_b[:128], pe_b[128:]], axis=1).astype(np.float32)

    shared = {
        "wqk": wqk.ravel(),
        "wtap": wtap.ravel(),
        "ones": np.ones(128 * 128, ml_dtypes.bfloat16),
        "bqk": np.ascontiguousarray(bqk).ravel(),
        "bq1": np.ascontiguousarray(bq1).ravel(),
        "peb": np.ascontiguousarray(pebh).ravel(),
    }
    in_maps = [
        {
            "xb": np.ascontiguousarray(xb16[r * SPC : (r + 1) * SPC]).ravel(),
            "xT": np.ascontiguousarray(xT16[r * SPC : (r + 1) * SPC]).ravel(),
            **shared,
        }
        for r in range(NCORES)
    ]

    if not _NC_CACHE:
        _NC_CACHE.append(_build())
    nc = _NC_CACHE[0]
    res = run_bass_kernel_spmd(nc, in_maps, list(range(NCORES)))

    full = np.empty((B, C, H, W), np.float32)
    for r in range(NCORES):
        o = res.results[r]["out"].astype(np.float32).reshape(SPC, 2, 128, N)
        full[r * SPC : (r + 1) * SPC] = o.reshape(SPC, C, H, W)
    return full


# revision 25
# speedup vs baseline: 1.4119x; 1.0299x over previous
"""Trainium2 Bass kernel for LinearAttention4 (self-contained).

Problem (per sample): x [256, 56, 56] fp32
  qk = elu(conv1x1(x; qk_w, qk_b)) + 1 ; q, k = split(qk)
  kv = k @ v.T ; num = q.T @ kv ; den = q.T @ sum(k) + eps ; attn = num/den
  out = attn + depthwise3x3(x; pe_w) + pe_b

Sharding: data-parallel over batch, 4 samples per core on 8 NeuronCores.

Dataflow (per sample, mostly bf16):
  A) DMA: x padded bf16 (qk rhs + conv taps), xT bf16 transposed+padded
     (kv rhs) -- all host-prepped layout/dtype casts, no host arithmetic.
  B) qk matmul bf16 -> psum; elu+1 = min(exp(z+b), max(z+b+1, 1)) split as
     ACT Exp -> e, DVE tensor_scalar -> lin, DVE tensor_tensor min -> q/k.
  C) kT via one DMA-transpose instruction (bf16, 27x [128,128] blocks);
     ksum = kT.T @ ones and kv = kT.T @ xT accumulated on PE.
  D) den = ksum.T @ q per span; fold [1,NP]->[116,29] via DMA, +eps,
     reciprocal -> bf16, unfold; bc = ones-row broadcast matmul; q *= bc.
  E) per (c-block, span): 9 bf16 diagonal tap matmuls (exact depthwise
     conv) + bf16 num matmul into one psum tile; ACT evacuates + pe_b,
     compacting to dense 56x56; bf16 out DMA, host casts to f32.
  Sample pairs are software-pipelined so PE always has the sibling
  sample's matmuls during the kT-transpose and reciprocal waits.
"""

import numpy as np
import ml_dtypes

import bass_rust
import concourse.bass as bass
import concourse.mybir as mybir
from concourse.tile import TileContext
from concourse.bass_utils import run_bass_kernel_spmd

F32 = mybir.dt.float32
BF16 = mybir.dt.bfloat16
FP8 = mybir.dt.float8e4

B, C, H, W = 32, 256, 56, 56
N = H * W  # 3136
NCORES = 8
SPC = B // NCORES  # 4
HP = H + 2  # 58
NP = HP * HP  # 3364
NPP = 27 * 128  # 3456: NP padded so DMA-transpose tiles divide evenly
SPAN = 8 * HP  # 464 cols per span (8 padded rows)
NCH = 7  # spans of 8 interior rows
EPS = 1e-6 * N  # den eps, rescaled because kv/ksum stay unscaled

# conv tap byte offsets into the padded grid, t = 3*dy + dx
OFFS = [HP * (dy - 1) + (dx - 1) for dy in range(3) for dx in range(3)]


def _split_multi_waits(nc, max_waits=1):
    """Walrus allows one SyncWait per instruction; hoist extras onto
    fresh same-engine NOPs placed immediately before (same semantics)."""
    for f in nc.m.functions:
        for blk in f.blocks:
            new_insts = []
            for ins in blk.instructions:
                si = ins.sync_info
                waits = list(si.on_wait) if si is not None else []
                if len(waits) > max_waits:
                    head, tail = waits[:-max_waits], waits[-max_waits:]
                    for w in head:
                        nop = mybir.InstNoOp(
                            name=f"Wsplit-{nc.next_id()}", engine=ins.engine,
                            ins=[], outs=[],
                        )
                        nop.sync_info = mybir.SyncInfo(on_wait=[w], on_update=[])
                        new_insts.append(nop)
                    ins.sync_info = mybir.SyncInfo(
                        on_wait=tail, on_update=list(si.on_update)
                    )
                new_insts.append(ins)
            blk.instructions = new_insts


def _pair_ap(tile_ap, base, delta, width=SPAN):
    """rhs view [128, 2, width] over a padded fp8 buffer: slot j reads at
    byte offset base + j*delta (delta may be 0 to read the same span twice)."""
    ap = tile_ap[:, base : base + width]
    ap2 = ap.copy()
    ap2.ap = bass_rust.VecI64Pair(
        [list(ap.ap[0]), [delta, 2], [1, width]]
    )
    return ap2


def _build(phases="ABMQCDETN"):
    nc = bass.Bass()
    # flat 1D DRAM params: PJRT/XLA may permute multi-dim parameter layouts
    xb_f = nc.declare_dram_parameter("xb", [SPC * 2 * 128 * NP], BF16, isOutput=False)
    xT_f = nc.declare_dram_parameter("xT", [SPC * 27 * 128 * 256], BF16, isOutput=False)
    wqk_f = nc.declare_dram_parameter("wqk", [128 * 2 * 2 * 128], BF16, isOutput=False)
    wtap_f = nc.declare_dram_parameter("wtap", [2 * 9 * 128 * 128], BF16, isOutput=False)
    ones_f = nc.declare_dram_parameter("ones", [128 * 128], BF16, isOutput=False)
    bqk_f = nc.declare_dram_parameter("bqk", [128 * 2], F32, isOutput=False)
    bq1_f = nc.declare_dram_parameter("bq1", [128 * 2], F32, isOutput=False)
    peb_f = nc.declare_dram_parameter("peb", [128 * 2], F32, isOutput=False)
    out_f = nc.declare_dram_parameter("out", [SPC * 2 * 128 * N], BF16, isOutput=True)

    xbs = xb_f[:].rearrange("(s c p n) -> s p c n", s=SPC, c=2, p=128)
    xTs = xT_f[:].rearrange("(s j p c) -> s p j c", s=SPC, j=27, p=128)
    out = out_f[:].rearrange("(s c p n) -> s c p n", s=SPC, c=2, p=128)

    Exp = mybir.ActivationFunctionType.Exp
    Ident = mybir.ActivationFunctionType.Identity
    ad, mx, mi, mu = (mybir.AluOpType.add, mybir.AluOpType.max,
                      mybir.AluOpType.min, mybir.AluOpType.mult)

    def span_start(ch):
        return HP * (1 + 8 * ch) + 1

    with TileContext(nc) as tc:
        with (
            tc.tile_pool(name="wp", bufs=1) as wp,
            tc.tile_pool(name="xpool", bufs=3) as xpool,
            tc.tile_pool(name="qkpool", bufs=3) as qkpool,
            tc.tile_pool(name="erpool", bufs=4) as erpool,
            tc.tile_pool(name="ktpool", bufs=2) as ktpool,
            tc.tile_pool(name="kvpool", bufs=2) as kvpool,
            tc.tile_pool(name="denpool", bufs=1) as denpool,
            tc.tile_pool(name="opool", bufs=2) as opool,
            tc.tile_pool(name="qkps", bufs=3, space="PSUM") as qkps,
            tc.tile_pool(name="kvps", bufs=1, space="PSUM") as kvps,
            tc.tile_pool(name="dbps", bufs=2, space="PSUM") as dbps,
            tc.tile_pool(name="bigps", bufs=2, space="PSUM") as bigps,
        ):
            w_qk = wp.tile([128, 512], BF16, name="w_qk")
            w_tap = wp.tile([128, 2 * 9 * 128], BF16, name="w_tap")
            w_ones = wp.tile([128, 128], BF16, name="w_ones")
            w_bqk = wp.tile([128, 2], F32, name="w_bqk")
            w_bq1 = wp.tile([128, 2], F32, name="w_bq1")
            w_peb = wp.tile([128, 2], F32, name="w_peb")
            dma = nc.default_dma_engine.dma_start
            dma(out=w_qk[:], in_=wqk_f[:].rearrange("(p q) -> p q", p=128))
            dma(
                out=w_tap[:].rearrange("p (c k m) -> p c k m", c=2, k=9),
                in_=wtap_f[:].rearrange("(c k p m) -> p c k m", c=2, k=9, p=128),
            )
            dma(out=w_ones[:], in_=ones_f[:].rearrange("(p j) -> p j", p=128))
            dma(out=w_bqk[:], in_=bqk_f[:].rearrange("(p c) -> p c", p=128))
            dma(out=w_bq1[:], in_=bq1_f[:].rearrange("(p c) -> p c", p=128))
            dma(out=w_peb[:], in_=peb_f[:].rearrange("(p c) -> p c", p=128))
            wtv = w_tap[:].rearrange("p (c k m) -> p c k m", c=2, k=9)

            def phase_A(s):
                xbt = xpool.tile([128, 2 * NP + 2], BF16, tag="xb", name="xbt")
                xTt = xpool.tile([128, 27 * 256], BF16, tag="xT", name="xTt")
                dma(out=xbt[:, 0 : 2 * NP].rearrange("p (c n) -> p c n", c=2), in_=xbs[s])
                dma(out=xTt[:].rearrange("p (j c) -> p j c", j=27), in_=xTs[s])
                nc.vector.memset(xbt[:, 2 * NP : 2 * NP + 2], 0)
                return {"xbt": xbt, "xTt": xTt}

            def phase_B_thunks(s, st):
                xbt = st["xbt"]
                q_t = qkpool.tile([128, NP], BF16, tag="q", name="q_t")
                k_t = qkpool.tile([128, NPP], BF16, tag="k", name="k_t")
                for t in (q_t, k_t):
                    nc.vector.memset(t[:, 0:59], 0)
                    pairs = t[:, 115 : 115 + 58 * 55 + 2]
                    pap = pairs.copy()
                    pap.ap = bass_rust.VecI64Pair(
                        [list(pairs.ap[0]), [58, 56], [1, 2]]
                    )
                    nc.vector.memset(pap, 0)
                    nc.vector.memset(
                        t[:, 3307 : (3307 + 57 if t is q_t else NPP)], 0
                    )
                thunks = []

                def b_group(mb, ch):
                    dst = q_t if mb == 0 else k_t
                    if True:
                        p1 = span_start(ch)
                        ps = None
                        ps = qkps.tile([128, SPAN], F32, tag="qkps", name="ps")
                        for cc in range(2):
                            nc.tensor.matmul(
                                ps[:],
                                w_qk[:].rearrange(
                                    "p (cc mb o) -> p cc mb o", cc=2, mb=2
                                )[:, cc, mb],
                                xbt[:, NP * cc + p1 : NP * cc + p1 + SPAN],
                                start=(cc == 0),
                                stop=(cc == 1),
                            )
                        e = erpool.tile([128, SPAN], BF16, tag="e", name="e")
                        lin = erpool.tile([128, SPAN], BF16, tag="lin", name="lin")
                        nc.scalar.activation(
                            e[:], ps[:], Exp, bias=w_bqk[:, mb : mb + 1], scale=1.0
                        )
                        nc.vector.tensor_scalar(
                            out=lin[:], in0=ps[:], scalar1=w_bq1[:, mb : mb + 1],
                            scalar2=1.0, op0=ad, op1=mx,
                        )
                        dst_v = dst[:, p1 : p1 + SPAN].rearrange(
                            "p (a b) -> p a b", b=HP
                        )[:, :, 0:56]
                        e_v = e[:].rearrange("p (a b) -> p a b", b=HP)[:, :, 0:56]
                        l_v = lin[:].rearrange("p (a b) -> p a b", b=HP)[:, :, 0:56]
                        nc.vector.tensor_tensor(dst_v, e_v, l_v, op=mi)

                for mb_ in range(2):
                    for ch_ in range(NCH):
                        thunks.append(lambda mb=mb_, ch=ch_: b_group(mb, ch))
                st.update({"q_t": q_t, "k_t": k_t})
                return thunks

            def phase_C(s, st):
                k_t, xTt = st["k_t"], st["xTt"]
                kT = ktpool.tile([128, 27 * 128], BF16, tag="kT", name="kT")
                nc.default_dma_engine.dma_start_transpose(
                    kT[:].rearrange("p (j d) -> p j d", j=27), k_t[:]
                )
                kvp = kvps.tile([128, 258], F32, tag="kvp", name="kvp")
                for j in range(27):
                    nc.tensor.matmul(
                        kvp[:, 256:257], kT[:, 128 * j : 128 * (j + 1)],
                        w_ones[:, 0:1],
                        start=(j == 0), stop=(j == 26),
                        skip_group_check=True,
                    )
                for j in range(27):
                    nc.tensor.matmul(
                        kvp[:, 0:256], kT[:, 128 * j : 128 * (j + 1)],
                        xTt[:, 256 * j : 256 * (j + 1)],
                        start=(j == 0), stop=(j == 26),
                        skip_group_check=True,
                    )
                ksumb = denpool.tile([128, 2], BF16, tag="ksumb", name="ksumb")
                kvb = kvpool.tile([128, 256], BF16, tag="kvb", name="kvb")
                nc.scalar.copy(ksumb[:, 0:1], kvp[:, 256:257])
                nc.scalar.copy(kvb[:], kvp[:, 0:256])
                st.update({"ksumb": ksumb, "kvb": kvb})

            def phase_D(s, st):
                q_t, ksumb = st["q_t"], st["ksumb"]
                den = denpool.tile([1, NP], F32, tag="den", name="den")
                for ch in range(NCH):
                    p1 = span_start(ch)
                    dp = dbps.tile([128, SPAN], F32, tag="dbps", name="dp")
                    nc.tensor.matmul(
                        dp[0:1, :], ksumb[:, 0:1],
                        q_t[:, p1 : p1 + SPAN],
                        start=True, stop=True,
                    )
                    nc.scalar.copy(den[:, p1 : p1 + SPAN], dp[0:1, :])
                recf = denpool.tile([116, 29], F32, tag="recf", name="recf")
                recb = denpool.tile([116, 29], BF16, tag="recb", name="recb")
                dma(out=recf[:], in_=den[:])
                nc.vector.tensor_scalar_add(recf[:], recf[:], EPS)
                with nc.allow_low_precision(
                    reason="bf16 reciprocal of den ~O(n): ~0.4% rel, fine"
                ):
                    nc.vector.reciprocal(recb[:], recf[:])
                rrow = denpool.tile([1, NP], BF16, tag="rrow", name="rrow")
                dma(out=rrow[:], in_=recb[:])
                for ch in range(NCH):
                    p1 = span_start(ch)
                    bc = dbps.tile([128, SPAN], F32, tag="dbps", name="bc")
                    nc.tensor.matmul(
                        bc[:], w_ones[0:1, :], rrow[:, p1 : p1 + SPAN],
                        start=True, stop=True,
                    )
                    nc.vector.tensor_tensor(
                        q_t[:, p1 : p1 + SPAN], q_t[:, p1 : p1 + SPAN], bc[:],
                        op=mu,
                    )

            def phase_E_thunks(s, st):
                xbt, q_t, kvb = st["xbt"], st["q_t"], st["kvb"]
                osbs = {}

                def e_group(cb, ch):
                    if ch == 0:
                        osbs[cb] = opool.tile([128, N], BF16, tag="osb", name="o_sb")
                    o_sb = osbs[cb]
                    if True:
                        p1 = span_start(ch)
                        pn = bigps.tile([128, SPAN], F32, tag="bigps", name="pn")
                        base = NP * cb + p1
                        for t in range(9):  # bf16 diagonal conv taps
                            nc.tensor.matmul(
                                pn[:], wtv[:, cb, t],
                                xbt[:, base + OFFS[t] : base + OFFS[t] + SPAN],
                                start=(t == 0), stop=False,
                                skip_group_check=True,
                            )
                        nc.tensor.matmul(
                            pn[:], kvb[:, 128 * cb : 128 * (cb + 1)],
                            q_t[:, p1 : p1 + SPAN],
                            start=False, stop=True,
                            skip_group_check=True,
                        )
                        nc.scalar.activation(
                            o_sb[:, 448 * ch : 448 * (ch + 1)].rearrange(
                                "p (y x) -> p y x", x=56
                            ),
                            pn[:].rearrange("p (y x) -> p y x", x=HP)[:, :, 0:56],
                            Ident, bias=w_peb[:, cb : cb + 1], scale=1.0,
                        )
                    if ch == NCH - 1:
                        dma(out=out[s, cb], in_=o_sb[:])

                return [lambda cb=cb_, ch=ch_: e_group(cb, ch)
                        for cb_ in range(2) for ch_ in range(NCH)]

            # software pipeline: pairs of samples run phase-interleaved, and
            # the previous pair's E-phase tap groups are woven into this
            # pair's DVE-bound B phase so the PE never starves (keeps p-state)
            pend_e = []
            for s0 in range(0, SPC, 2):
                s1 = s0 + 1
                stA = phase_A(s0)
                stB = phase_A(s1)
                bth = phase_B_thunks(s0, stA) + phase_B_thunks(s1, stB)
                bi, ei = 0, 0
                while bi < len(bth) or ei < len(pend_e):
                    for _ in range(2):
                        if bi < len(bth):
                            bth[bi]()
                            bi += 1
                    if ei < len(pend_e):
                        pend_e[ei]()
                        ei += 1
                phase_C(s0, stA)
                phase_C(s1, stB)
                phase_D(s0, stA)
                phase_D(s1, stB)
                pend_e = phase_E_thunks(s0, stA) + phase_E_thunks(s1, stB)
            for th in pend_e:
                th()

    _split_multi_waits(nc)
    return nc


_NC_CACHE = []


def kernel(x, qk_w, qk_b, pe_w, pe_b):
    x = np.asarray(x, np.float32)
    qk_w = np.asarray(qk_w, np.float32)
    qk_b = np.asarray(qk_b, np.float32)
    pe_w = np.asarray(pe_w, np.float32)
    pe_b = np.asarray(pe_b, np.float32)

    # host prep: zero-padded 58x58 layout + dtype casts (no arithmetic)
    xpad = np.zeros((B, 2, 128, HP, HP), np.float32)
    xpad[:, :, :, 1 : H + 1, 1 : W + 1] = x.reshape(B, 2, 128, H, W)
    xpad = xpad.reshape(B, 2, 128, NP)
    xb16 = xpad.astype(ml_dtypes.bfloat16)
    xT = np.zeros((B, NPP, 256), np.float32)
    xT[:, :NP] = xpad.reshape(B, 256, NP).transpose(0, 2, 1)
    xT16 = xT.reshape(B, 27, 128, 256).astype(ml_dtypes.bfloat16)

    wqk = np.ascontiguousarray(
        qk_w.T.reshape(2, 128, 2, 128).transpose(1, 0, 2, 3)
    ).astype(ml_dtypes.bfloat16)  # [p, cc, mb, o]

    w0 = pe_w[:, 0].reshape(256, 9)
    wtap = np.zeros((2, 9, 128, 128), ml_dtypes.bfloat16)
    idx = np.arange(128)
    for cb in range(2):
        sl = slice(128 * cb, 128 * (cb + 1))
        for t in range(9):
            wtap[cb, t, idx, idx] = w0[sl, t]

    bqk = np.stack([qk_b[:128], qk_b[128:]], axis=1).astype(np.float32)
    bq1 = bqk + 1.0
    pebh = np.stack([pe_b[:128], pe_b[128:]], axis=1).astype(np.float32)

    shared = {
        "wqk": wqk.ravel(),
        "wtap": wtap.ravel(),
        "ones": np.ones(128 * 128, ml_dtypes.bfloat16),
        "bqk": np.ascontiguousarray(bqk).ravel(),
        "bq1": np.ascontiguousarray(bq1).ravel(),
        "peb": np.ascontiguousarray(pebh).ravel(),
    }
    in_maps = [
        {
            "xb": np.ascontiguousarray(xb16[r * SPC : (r + 1) * SPC]).ravel(),
            "xT": np.ascontiguousarray(xT16[r * SPC : (r + 1) * SPC]).ravel(),
            **shared,
        }
        for r in range(NCORES)
    ]

    if not _NC_CACHE:
        _NC_CACHE.append(_build())
    nc = _NC_CACHE[0]
    res = run_bass_kernel_spmd(nc, in_maps, list(range(NCORES)))

    full = np.empty((B, C, H, W), np.float32)
    for r in range(NCORES):
        o = res.results[r]["out"].astype(np.float32).reshape(SPC, 2, 128, N)
        full[r * SPC : (r + 1) * SPC] = o.reshape(SPC, C, H, W)
    return full
